# revision 14
# baseline (speedup 1.0000x reference)
"""Trainium2 Bass kernel for AdvancedLSTMCell (B=16384, IN=512, H=1024).

Data-parallel over batch across 8 NeuronCores (2048 rows each). All compute is
done in a transposed layout (features on partitions, batch on the free dim) so
the batch dim streams through the PE array as matmul moving columns:

  gatesT[4H, b] = W_gates^T.T-free @ combinedT    (K = IN+H = 1536)
  c = sigmoid(f)*c_prev + sigmoid(i)*tanh(g); h_pre = sigmoid(o)*tanh(c)
  logits[1, b] = W_a ap h_pre  -> exp -> local sum -> AllReduce(add) over the
  8 cores -> attn = exp/S_global broadcast over partitions
  highwayT = [W_ht|W_hg]^T @ xT; h = gate*t + (1-gate)*h_pre*attn

Matmul inputs are cast to bf16 (fp32 PSUM accumulation); everything elementwise
stays fp32. Host-side prep only reshapes/transposes/casts operands.
"""

import numpy as np
import ml_dtypes

import concourse.bass as bass
from concourse import bacc
import concourse.mybir as mybir
import concourse.tile as tile
from concourse.bass_utils import run_bass_kernel_spmd

F32 = mybir.dt.float32
BF16 = mybir.dt.bfloat16
AF = mybir.ActivationFunctionType
ALU = mybir.AluOpType

B, IN, H = 16384, 512, 1024
NCORES = 8
BL = B // NCORES          # 2048 rows per core
NB = 4                    # batch chunks of 512 columns
BC = BL // NB             # 512
KG = (IN + H) // 128      # 12 k-tiles for the gates GEMM
KH = IN // 128            # 4 k-tiles for the highway GEMM
NJ = H // 128             # 8 h-blocks

_cached = {}


def build_program(reps: int = 1, single: bool = False):
    nc = bacc.Bacc("TRN2", target_bir_lowering=False, debug=False,
                   num_devices=1 if single else NCORES)

    xt = nc.dram_tensor("xt", [KH, 128, BL], BF16, kind="ExternalInput").ap()
    ht = nc.dram_tensor("ht", [NJ, 128, BL], BF16, kind="ExternalInput").ap()
    ct = nc.dram_tensor("ct", [NJ, 128, BL], F32, kind="ExternalInput").ap()
    wg = nc.dram_tensor("wg", [NJ, KG, 128, 512], BF16, kind="ExternalInput").ap()
    wh = nc.dram_tensor("wh", [NJ, KH, 128, 256], BF16, kind="ExternalInput").ap()
    wa = nc.dram_tensor("wa", [128, NJ], BF16, kind="ExternalInput").ap()
    bg = nc.dram_tensor("bg", [128, 4 * NJ], F32, kind="ExternalInput").ap()
    bh = nc.dram_tensor("bh", [128, 2 * NJ], F32, kind="ExternalInput").ap()
    ba = nc.dram_tensor("ba", [1, 1], F32, kind="ExternalInput").ap()
    ho = nc.dram_tensor("ho", [NJ, 128, BL], F32, kind="ExternalOutput").ap()
    co = nc.dram_tensor("co", [NJ, 128, BL], F32, kind="ExternalOutput").ap()

    with tile.TileContext(nc) as tc:
        with (
            tc.tile_pool(name="const", bufs=1) as const,
            tc.tile_pool(name="wgp", bufs=12) as wgp,
            tc.tile_pool(name="whp", bufs=8) as whp,
            tc.tile_pool(name="cpp", bufs=2) as cpp,
            tc.tile_pool(name="gact", bufs=2) as gact,
            tc.tile_pool(name="cbuf", bufs=2) as cbuf,
            tc.tile_pool(name="hwp", bufs=2) as hwp,
            tc.tile_pool(name="small", bufs=1) as small,
            tc.tile_pool(name="ps", bufs=8, space="PSUM") as psp,
            tc.tile_pool(name="dram", bufs=2, space="DRAM") as dramp,
        ):
            # ---- resident tensors ----
            xt_sb = const.tile([128, KH, BL], BF16, tag="xt_sb")
            ht_sb = const.tile([128, NJ, BL], BF16, tag="ht_sb")
            hpre = const.tile([128, NJ, BL], BF16, tag="hpre")
            attn_bc = const.tile([128, BL], BF16, tag="attn_bc")
            wa_sb = const.tile([128, NJ], BF16, tag="wa_sb")
            bg_sb = const.tile([128, 4 * NJ], F32, tag="bg_sb")
            bh_sb = const.tile([128, 2 * NJ], F32, tag="bh_sb")
            ba_sb = const.tile([1, 1], F32, tag="ba_sb")

            for k in range(KH):
                nc.scalar.dma_start(xt_sb[:, k, :], xt[k])
            for j in range(NJ):
                nc.scalar.dma_start(ht_sb[:, j, :], ht[j])
            nc.scalar.dma_start(wa_sb, wa)
            nc.scalar.dma_start(bg_sb, bg)
            nc.scalar.dma_start(bh_sb, bh)
            nc.scalar.dma_start(ba_sb, ba)

            for _ in range(reps):
                # ================= Phase A: gates + cell update =============
                for j in range(NJ):
                    cp = cpp.tile([128, BL], F32, tag="cp")
                    nc.scalar.dma_start(cp, ct[j])
                    c_t = cbuf.tile([128, BL], F32, tag="c_t")
                    for b4 in range(NB):
                        sl = slice(b4 * BC, (b4 + 1) * BC)
                        pg = [psp.tile([128, BC], F32, tag="ps", name=f"pg{j}_{b4}_{g}")
                              for g in range(4)]
                        for kt in range(KG):
                            w = wgp.tile([128, 512], BF16, tag="wg")
                            nc.sync.dma_start(w, wg[j, kt])
                            if kt < KH:
                                rhs = xt_sb[:, kt, sl]
                            else:
                                rhs = ht_sb[:, kt - KH, sl]
                            for g in range(4):
                                nc.tensor.matmul(
                                    pg[g], w[:, g * 128:(g + 1) * 128], rhs,
                                    start=(kt == 0), stop=(kt == KG - 1),
                                )
                        i_t = gact.tile([128, BC], F32, tag="i_t")
                        f_t = gact.tile([128, BC], F32, tag="f_t")
                        o_t = gact.tile([128, BC], F32, tag="o_t")
                        g_t = gact.tile([128, BC], F32, tag="g_t")
                        ig = gact.tile([128, BC], F32, tag="ig")
                        nc.scalar.activation(i_t, pg[0], AF.Sigmoid,
                                             bias=bg_sb[:, 0 * NJ + j:0 * NJ + j + 1])
                        nc.scalar.activation(f_t, pg[1], AF.Sigmoid,
                                             bias=bg_sb[:, 1 * NJ + j:1 * NJ + j + 1])
                        nc.scalar.activation(o_t, pg[2], AF.Sigmoid,
                                             bias=bg_sb[:, 2 * NJ + j:2 * NJ + j + 1])
                        nc.scalar.activation(g_t, pg[3], AF.Tanh,
                                             bias=bg_sb[:, 3 * NJ + j:3 * NJ + j + 1])
                        nc.vector.tensor_mul(ig, i_t, g_t)
                        nc.vector.tensor_mul(c_t[:, sl], f_t, cp[:, sl])
                        nc.vector.tensor_add(c_t[:, sl], c_t[:, sl], ig)
                        nc.scalar.activation(cp[:, sl], c_t[:, sl], AF.Tanh)
                        nc.vector.tensor_mul(hpre[:, j, sl], o_t, cp[:, sl])
                    nc.scalar.dma_start(co[j], c_t)

                # ================= Phase B: softmax over batch ==============
                lps = [psp.tile([128, BC], F32, tag="ps", name=f"lps{b4}")
                       for b4 in range(NB)]
                for j in range(NJ):
                    for b4 in range(NB):
                        sl = slice(b4 * BC, (b4 + 1) * BC)
                        nc.tensor.matmul(
                            lps[b4][:1, :], wa_sb[:, j:j + 1], hpre[:, j, sl],
                            start=(j == 0), stop=(j == NJ - 1),
                        )
                exp_l = small.tile([1, BL], F32, tag="exp_l")
                for b4 in range(NB):
                    sl = slice(b4 * BC, (b4 + 1) * BC)
                    nc.scalar.activation(exp_l[:, sl], lps[b4][:1, :], AF.Exp,
                                         bias=ba_sb)
                s_loc = small.tile([1, 1], F32, tag="s_loc")
                nc.vector.reduce_sum(s_loc, exp_l, axis=mybir.AxisListType.X)
                s_glob = small.tile([1, 1], F32, tag="s_glob")
                if single:
                    nc.vector.tensor_copy(s_glob, s_loc)
                else:
                    cc_in = dramp.tile([1, 1], F32, tag="cc_in")
                    cc_out = dramp.tile([1, 1], F32, tag="cc_out")
                    nc.sync.dma_start(cc_in, s_loc)
                    nc.gpsimd.collective_compute(
                        "AllReduce", ALU.add,
                        replica_groups=[list(range(NCORES))],
                        ins=[cc_in.opt()],
                        outs=[cc_out.opt()],
                    )
                    nc.sync.dma_start(s_glob, cc_out)
                r_s = small.tile([1, 1], F32, tag="r_s")
                nc.vector.reciprocal(r_s, s_glob)
                nc.vector.tensor_scalar_mul(exp_l, exp_l, r_s)
                attn16 = small.tile([1, BL], BF16, tag="attn16")
                nc.vector.tensor_copy(attn16, exp_l)
                nc.gpsimd.partition_broadcast(attn_bc, attn16)

                # ================= Phase C: highway + merge =================
                for j in range(NJ):
                    hout = hwp.tile([128, BL], F32, tag="hout")
                    for bp in range(2):
                        pt = [psp.tile([128, BC], F32, tag="ps", name=f"pt{j}_{bp}_{b2}")
                              for b2 in range(2)]
                        pss = [psp.tile([128, BC], F32, tag="ps", name=f"pss{j}_{bp}_{b2}")
                               for b2 in range(2)]
                        for kt in range(KH):
                            w = whp.tile([128, 256], BF16, tag="wh")
                            nc.sync.dma_start(w, wh[j, kt])
                            for b2 in range(2):
                                b4 = bp * 2 + b2
                                sl = slice(b4 * BC, (b4 + 1) * BC)
                                nc.tensor.matmul(pt[b2], w[:, :128],
                                                 xt_sb[:, kt, sl],
                                                 start=(kt == 0), stop=(kt == KH - 1))
                                nc.tensor.matmul(pss[b2], w[:, 128:],
                                                 xt_sb[:, kt, sl],
                                                 start=(kt == 0), stop=(kt == KH - 1))
                        t_t = hwp.tile([128, 2 * BC], F32, tag="t_t")
                        s_t = hwp.tile([128, 2 * BC], F32, tag="s_t")
                        hh = hwp.tile([128, 2 * BC], F32, tag="hh")
                        for b2 in range(2):
                            nc.scalar.activation(t_t[:, b2 * BC:(b2 + 1) * BC],
                                                 pt[b2], AF.Identity,
                                                 bias=bh_sb[:, j:j + 1])
                            nc.scalar.activation(s_t[:, b2 * BC:(b2 + 1) * BC],
                                                 pss[b2], AF.Sigmoid,
                                                 bias=bh_sb[:, NJ + j:NJ + j + 1])
                        slp = slice(bp * 2 * BC, (bp + 1) * 2 * BC)
                        nc.vector.tensor_mul(hh, hpre[:, j, slp], attn_bc[:, slp])
                        nc.vector.tensor_sub(t_t, t_t, hh)
                        nc.vector.tensor_mul(t_t, s_t, t_t)
                        nc.vector.tensor_add(hout[:, slp], t_t, hh)
                        nc.scalar.dma_start(ho[j][:, slp], hout[:, slp])
    nc.compile()
    return nc


def _prep_inputs(x, h_prev, c_prev, W_i, b_i, W_f, b_f, W_o, b_o, W_g, b_g,
                 W_a, b_a, W_ht, b_ht, W_hg, b_hg):
    bf16 = ml_dtypes.bfloat16
    f32 = np.float32

    W_gates = np.concatenate([W_i, W_f, W_o, W_g], axis=0)          # [4H, IN+H]
    # wg[j, kt] = [128 k, 512] with the 4 gate blocks {j, NJ+j, 2NJ+j, 3NJ+j}
    wg = (W_gates.T.astype(f32)
          .reshape(KG, 128, 4, NJ, 128)     # [kt, k, g, j, m]
          .transpose(3, 0, 1, 2, 4)         # [j, kt, k, g, m]
          .reshape(NJ, KG, 128, 512)
          .astype(bf16))
    W_h = np.concatenate([W_ht, W_hg], axis=0)                      # [2H, IN]
    wh = (W_h.T.astype(f32)
          .reshape(KH, 128, 2, NJ, 128)
          .transpose(3, 0, 1, 2, 4)
          .reshape(NJ, KH, 128, 256)
          .astype(bf16))
    wa = np.ascontiguousarray(
        np.asarray(W_a, f32).reshape(NJ, 128).T).astype(bf16)       # [128, NJ]
    bgp = (np.concatenate([b_i, b_f, b_o, b_g])
           .reshape(4, NJ, 128).transpose(2, 0, 1).reshape(128, 4 * NJ))
    bhp = (np.concatenate([b_ht, b_hg])
           .reshape(2, NJ, 128).transpose(2, 0, 1).reshape(128, 2 * NJ))
    bap = np.asarray(b_a, f32).reshape(1, 1)

    x = np.asarray(x, f32)
    h_prev = np.asarray(h_prev, f32)
    c_prev = np.asarray(c_prev, f32)

    in_maps = []
    for c in range(NCORES):
        rows = slice(c * BL, (c + 1) * BL)
        xtc = np.ascontiguousarray(x[rows].T).reshape(KH, 128, BL).astype(bf16)
        htc = np.ascontiguousarray(h_prev[rows].T).reshape(NJ, 128, BL).astype(bf16)
        ctc = np.ascontiguousarray(c_prev[rows].T).reshape(NJ, 128, BL)
        in_maps.append({
            "xt": xtc, "ht": htc, "ct": ctc,
            "wg": wg, "wh": wh, "wa": wa,
            "bg": np.ascontiguousarray(bgp, f32),
            "bh": np.ascontiguousarray(bhp, f32),
            "ba": bap,
        })
    return in_maps


def kernel(**inputs):
    if "nc" not in _cached:
        _cached["nc"] = build_program()
    nc = _cached["nc"]
    in_maps = _prep_inputs(**inputs)
    res = run_bass_kernel_spmd(nc, in_maps, core_ids=list(range(NCORES)))
    h = np.empty((B, H), np.float32)
    c = np.empty((B, H), np.float32)
    for ci, out in enumerate(res.results):
        rows = slice(ci * BL, (ci + 1) * BL)
        h[rows] = out["ho"].reshape(H, BL).T
        c[rows] = out["co"].reshape(H, BL).T
    return (h, c)
